# revision 16
# baseline (speedup 1.0000x reference)
"""Causal self-attention (B=2, T=2048, dim=2048, H=16, D=128) on 8 trn2 NeuronCores.

Sharding: data-parallel over batch (2 groups of 4 cores), tensor-parallel over
heads within a group (4 heads/core).  Each core computes its heads' QKV
projection (x @ Wqkv_part^T), RoPE, causal attention, and a partial output
projection against its W_proj column block; the host sums the 4 partials per
batch element.

Schedule (single PE instruction stream, tile framework inserts semaphores):
  - startup: x window-0 and W_qkv stream in 128-row chunks so the first
    matmul issues ~2us in; window-0 q-tiles accumulate c-outer across 4 PSUM
    banks so the PE tracks the arriving chunks.
  - QKV windows carry RoPE (DVE) inline per window; rotate-half via
    SBUF-to-SBUF DMA on the sync queue.
  - attention per (head, query-window) block: S^T = k.T@q tiles, exp on
    Act, PV + ones-rowsum accumulate in PSUM; S matmuls run 3 tiles ahead
    of PV so the PE never waits on exp.  Softmax normalization is deferred:
    reciprocal (DVE) -> K=1 broadcast matmul into the (dead) rowsum bank ->
    one DVE multiply; the norm of each block is emitted inside the next
    block / proj so its PE bubble is covered by independent matmuls.
  - output projection for window w-1 is emitted after window w's attention
    blocks; y partials stored bf16 (host sums in f32).
"""

import os

import numpy as np
import ml_dtypes

import concourse.bass as bass
import concourse.bacc as bacc
import concourse.tile as tile
import concourse.mybir as mybir
from concourse import bass_utils

BF16 = mybir.dt.bfloat16
F32 = mybir.dt.float32

B, T, DIM = 2, 2048, 2048
H, D = 16, 128
HL = 4                   # heads per core
NCORES = 8
E = 3 * HL * D           # 1536 = per-core qkv output rows
NCHUNK = DIM // 128      # 16 contraction chunks
NW = T // 512            # 4 query windows
NTT = T // 128           # 16 token tiles
SCALE = 1.0 / float(np.sqrt(D))

_CACHE = {}
LAST_RESULTS = None


def _build_module():
    nc = bacc.Bacc("TRN2", target_bir_lowering=False, debug=False)
    xT = nc.dram_tensor("xT", (DIM, T), BF16, kind="ExternalInput")
    wqkvT = nc.dram_tensor("wqkvT", (DIM, E), BF16, kind="ExternalInput")
    wpT = nc.dram_tensor("wpT", (HL * D, DIM), BF16, kind="ExternalInput")
    cosT = nc.dram_tensor("cosT", (D, T), BF16, kind="ExternalInput")
    sinTs = nc.dram_tensor("sinTs", (D, T), BF16, kind="ExternalInput")
    tri = nc.dram_tensor("tri", (128, 128), BF16, kind="ExternalInput")
    ones = nc.dram_tensor("ones", (128, 1), BF16, kind="ExternalInput")
    ones_row = nc.dram_tensor("ones_row", (1, 128), BF16, kind="ExternalInput")
    y = nc.dram_tensor("y", (T, DIM), BF16, kind="ExternalOutput")

    Exp = mybir.ActivationFunctionType.Exp

    xT_v = xT.rearrange("(c p) t -> p c t", p=128)
    wqkvT_v = wqkvT.rearrange("(c p) e -> p c e", p=128)

    with tile.TileContext(nc) as tc:
        with (
            tc.tile_pool(name="const", bufs=1) as cpool,
            tc.tile_pool(name="xp", bufs=2) as xpool,
            tc.tile_pool(name="rotp", bufs=4) as rotpool,
            tc.tile_pool(name="ptp", bufs=4) as ptpool,
            tc.tile_pool(name="yp", bufs=4) as ypool,
            tc.tile_pool(name="rcpp", bufs=2) as rcppool,
            tc.tile_pool(name="bcp", bufs=2) as bcpool,
            tc.tile_pool(name="ps", bufs=3, space="PSUM") as pspool,
            tc.tile_pool(name="otp", bufs=3, space="PSUM") as otpool,
            tc.tile_pool(name="rsp", bufs=2, space="PSUM") as rspool,
        ):
            # persistent SBUF
            w_sb = cpool.tile([128, NCHUNK, E], BF16, tag="w")
            wp_sb = cpool.tile([128, HL, DIM], BF16, tag="wp")
            cos_sb = cpool.tile([128, T], BF16, tag="cos")
            sin_sb = cpool.tile([128, T], BF16, tag="sin")
            tri_sb = cpool.tile([128, 128], BF16, tag="tri")
            ones_sb = cpool.tile([128, 1], BF16, tag="ones")
            onesr_sb = cpool.tile([1, 128], BF16, tag="onesr")
            q_sb = cpool.tile([128, HL * T], BF16, tag="q")
            k_sb = cpool.tile([128, HL * T], BF16, tag="k")
            v_sb = cpool.tile([128, NTT * HL * D], BF16, tag="v")
            o_sb = cpool.tile([128, HL * T], BF16, tag="o")

            # ---- startup DMAs: window-0 x chunks interleaved with W chunks
            xb0 = xpool.tile([128, NCHUNK, 512], BF16, tag="x")
            for c in range(NCHUNK):
                nc.sync.dma_start(xb0[:, c, :], xT_v[:, c, 0:512])
                nc.sync.dma_start(w_sb[:, c, 0:512], wqkvT_v[:, c, 0:512])
            for grp in (1, 2):
                for c in range(NCHUNK):
                    nc.sync.dma_start(
                        w_sb[:, c, grp * 512 : (grp + 1) * 512],
                        wqkvT_v[:, c, grp * 512 : (grp + 1) * 512],
                    )
            nc.sync.dma_start(cos_sb[:], cosT[:, :])
            nc.sync.dma_start(sin_sb[:], sinTs[:, :])
            nc.sync.dma_start(tri_sb[:], tri[:, :])
            nc.sync.dma_start(ones_sb[:], ones[:, :])
            nc.sync.dma_start(onesr_sb[:], ones_row[:, :])

            def rope(dst, h, w):
                """RoPE in place on dst[:, h*T + w*512 : ...+512] (d on partitions)."""
                sl = slice(h * T + w * 512, h * T + (w + 1) * 512)
                ws = slice(w * 512, (w + 1) * 512)
                rot = rotpool.tile([128, 512], BF16, tag="rot")
                nc.sync.dma_start(rot[0:64, :], dst[64:128, sl])
                nc.sync.dma_start(rot[64:128, :], dst[0:64, sl])
                nc.vector.tensor_mul(rot[:], rot[:], sin_sb[:, ws])
                nc.vector.tensor_mul(dst[:, sl], dst[:, sl], cos_sb[:, ws])
                nc.vector.tensor_add(dst[:, sl], dst[:, sl], rot[:])

            # ---- window-0 QKV.  Pass A: first 3 q tiles, c-outer over 3
            # concurrent PSUM groups so the PE consumes W/x chunks as they land.
            psA = []
            for j in range(3):
                ps = pspool.tile([128, 512], F32, tag="ps")
                psA.append(ps)
            for c in range(NCHUNK):
                for j in range(3):
                    nc.tensor.matmul(
                        psA[j][:],
                        w_sb[:, c, j * 128 : (j + 1) * 128],
                        xb0[:, c, :],
                        start=(c == 0),
                        stop=(c == NCHUNK - 1),
                    )
            for j in range(3):
                nc.scalar.copy(q_sb[:, j * T : j * T + 512], psA[j][:])
                rope(q_sb, j, 0)
            # Pass B: q3 + k tiles then v tiles (weights fully resident by now).
            for grp, j, dst in (
                [(0, 3, q_sb)] + [(1, j, k_sb) for j in range(HL)]
            ):
                ps = pspool.tile([128, 512], F32, tag="ps")
                base = grp * 512 + j * 128
                for c in range(NCHUNK):
                    nc.tensor.matmul(
                        ps[:],
                        w_sb[:, c, base : base + 128],
                        xb0[:, c, :],
                        start=(c == 0),
                        stop=(c == NCHUNK - 1),
                    )
                nc.scalar.copy(dst[:, j * T : j * T + 512], ps[:])
                rope(dst, j, 0)
            for ttl in range(4):
                ps = pspool.tile([128, 512], F32, tag="ps")
                for c in range(NCHUNK):
                    nc.tensor.matmul(
                        ps[:],
                        xb0[:, c, ttl * 128 : (ttl + 1) * 128],
                        w_sb[:, c, 1024:1536],
                        start=(c == 0),
                        stop=(c == NCHUNK - 1),
                    )
                nc.scalar.copy(v_sb[:, ttl * 512 : (ttl + 1) * 512], ps[:])

            # ---- windows 1..3 QKV + inline RoPE
            for w in range(1, NW):
                xb = xpool.tile([128, NCHUNK, 512], BF16, tag="x")
                nc.sync.dma_start(xb[:], xT_v[:, :, w * 512 : (w + 1) * 512])
                for grp, dst in ((0, q_sb), (1, k_sb)):
                    for j in range(HL):
                        ps = pspool.tile([128, 512], F32, tag="ps")
                        base = grp * 512 + j * 128
                        for c in range(NCHUNK):
                            nc.tensor.matmul(
                                ps[:],
                                w_sb[:, c, base : base + 128],
                                xb[:, c, :],
                                start=(c == 0),
                                stop=(c == NCHUNK - 1),
                            )
                        nc.scalar.copy(
                            dst[:, j * T + w * 512 : j * T + (w + 1) * 512], ps[:]
                        )
                for ttl in range(4):
                    ttg = w * 4 + ttl
                    ps = pspool.tile([128, 512], F32, tag="ps")
                    for c in range(NCHUNK):
                        nc.tensor.matmul(
                            ps[:],
                            xb[:, c, ttl * 128 : (ttl + 1) * 128],
                            w_sb[:, c, 1024:1536],
                            start=(c == 0),
                            stop=(c == NCHUNK - 1),
                        )
                    nc.scalar.copy(v_sb[:, ttg * 512 : (ttg + 1) * 512], ps[:])
                if w == 1:
                    nc.sync.dma_start(
                        wp_sb[:], wpT.rearrange("(h p) n -> p h n", p=128)
                    )

            # RoPE for windows 1..3 batched here: keeps the QKV windows free
            # of concurrent DVE/DMA traffic (multi-engine overlap trips the
            # PE duty throttle), and overlaps the small early attention
            # windows instead.  Window-major order unblocks attention blocks
            # in the order they are emitted.
            for w in range(1, NW):
                for dst in (q_sb, k_sb):
                    for j in range(HL):
                        rope(dst, j, w)

            # ---- attention + proj, software pipelined -------------------
            def emit_norm(pend):
                """Deferred softmax normalization of a finished block."""
                h, w, oT, rsb = pend
                rcp = rcppool.tile([1, 512], BF16, tag="rcp")
                with nc.allow_low_precision(reason="bf16 softmax denom, ~0.4% rel"):
                    nc.vector.reciprocal(rcp[:], rsb[0:1, :])
                # broadcast rcp across partitions on gpsimd: keeps the chain
                # off PE/Act/DVE (an Act-side copy queues behind the exp
                # backlog and stalls the oT bank rotation)
                bc = bcpool.tile([128, 512], BF16, tag="bc")
                nc.gpsimd.partition_broadcast(bc[:], rcp[:], channels=128)
                nc.vector.tensor_mul(
                    o_sb[:, h * T + w * 512 : h * T + (w + 1) * 512], oT[:], bc[:]
                )

            def attn_block(h, w, pending):
                hq = h * T
                nkt = 4 * w + 4
                oT = otpool.tile([128, 512], F32, tag="ot")
                rsb = rspool.tile([1, 512], F32, tag="rs")
                pts = [None] * nkt
                geom = []
                for kt in range(nkt):
                    if kt < 4 * w:
                        geom.append((512 * w, 512, False))
                    else:
                        geom.append((128 * kt, 512 * (w + 1) - 128 * kt, True))

                def emit_S(kt):
                    q0, n, diag = geom[kt]
                    st = pspool.tile([128, 512], F32, tag="ps")
                    nc.tensor.matmul(
                        st[:, :n],
                        k_sb[:, hq + kt * 128 : hq + (kt + 1) * 128],
                        q_sb[:, hq + q0 : hq + q0 + n],
                        start=True,
                        stop=True,
                    )
                    pt = ptpool.tile([128, 512], BF16, tag="pt")
                    nc.scalar.activation(pt[:, :n], st[:, :n], Exp, bias=0.0, scale=SCALE)
                    if diag:
                        nc.vector.tensor_mul(pt[:, 0:128], pt[:, 0:128], tri_sb[:])
                    pts[kt] = pt

                for kt in range(min(3, nkt)):
                    emit_S(kt)
                if pending is not None:
                    emit_norm(pending)
                for kt in range(nkt):
                    if kt + 3 < nkt:
                        emit_S(kt + 3)
                    q0, n, diag = geom[kt]
                    off = q0 - 512 * w
                    pt = pts[kt]
                    nc.tensor.matmul(
                        oT[:, off:512],
                        v_sb[:, kt * 512 + h * 128 : kt * 512 + (h + 1) * 128],
                        pt[:, :n],
                        start=(kt == 0),
                        stop=(kt == nkt - 1),
                    )
                    nc.tensor.matmul(
                        rsb[0:1, off:512],
                        ones_sb[:],
                        pt[:, :n],
                        start=(kt == 0),
                        stop=(kt == nkt - 1),
                    )
                return (h, w, oT, rsb)

            def proj_tile(tt, pending):
                for nw2 in range(DIM // 512):
                    yps = pspool.tile([128, 512], F32, tag="ps")
                    for hh in range(HL):
                        nc.tensor.matmul(
                            yps[:],
                            o_sb[:, hh * T + tt * 128 : hh * T + (tt + 1) * 128],
                            wp_sb[:, hh, nw2 * 512 : (nw2 + 1) * 512],
                            start=(hh == 0),
                            stop=(hh == HL - 1),
                        )
                    if pending is not None:
                        emit_norm(pending)
                        pending = None
                    ysb = ypool.tile([128, 512], BF16, tag="y")
                    nc.vector.tensor_copy(ysb[:], yps[:])
                    nc.sync.dma_start(
                        y[tt * 128 : (tt + 1) * 128, nw2 * 512 : (nw2 + 1) * 512],
                        ysb[:],
                    )
                return pending

            # attention blocks with the previous window's projection token
            # tiles interleaved one per block (spreads DVE copies + y DMAs
            # and keeps independent PE work at every block boundary)
            pending = None
            for w in range(NW):
                for h in range(HL):
                    pending = attn_block(h, w, pending)
                    if w >= 1:
                        pending = proj_tile(4 * (w - 1) + h, pending)
            for tt in range(4 * (NW - 1), 4 * NW):
                pending = proj_tile(tt, pending)
            assert pending is None

    nc.compile()
    return nc


def _rope_tables():
    inv_freq = (
        1.0 / (10000.0 ** (np.arange(0, D, 2, dtype=np.float32) / np.float32(D)))
    ).astype(np.float32)
    tpos = np.arange(T, dtype=np.float32)
    freqs = tpos[:, None] * inv_freq[None, :]
    emb = np.concatenate([freqs, freqs], axis=1)  # (T, D)
    cos = np.cos(emb).astype(np.float32)
    sin = np.sin(emb).astype(np.float32)
    cosT = np.ascontiguousarray(cos.T)  # (D, T)
    sinTs = np.ascontiguousarray(sin.T)
    sinTs[0:64] *= -1.0  # fold rotate_half sign
    return (
        cosT.astype(ml_dtypes.bfloat16),
        sinTs.astype(ml_dtypes.bfloat16),
    )


def make_in_maps(x, W_qkv, W_proj):
    cosT, sinTs = _rope_tables()
    tri = (np.arange(128)[None, :] >= np.arange(128)[:, None]).astype(
        ml_dtypes.bfloat16
    )
    tri = np.ascontiguousarray(tri)
    ones = np.ones((128, 1), dtype=ml_dtypes.bfloat16)
    in_maps = []
    for c in range(NCORES):
        b, g = divmod(c, 4)
        Wq = W_qkv[512 * g : 512 * (g + 1)]
        Wk = W_qkv[2048 + 512 * g : 2048 + 512 * (g + 1)]
        Wv = W_qkv[4096 + 512 * g : 4096 + 512 * (g + 1)]
        Wc = np.concatenate([Wq, Wk, Wv], axis=0)  # (1536, 2048)
        in_maps.append(
            {
                "xT": np.ascontiguousarray(x[b].T).astype(ml_dtypes.bfloat16),
                "wqkvT": np.ascontiguousarray(Wc.T).astype(ml_dtypes.bfloat16),
                "wpT": np.ascontiguousarray(
                    W_proj[:, 512 * g : 512 * (g + 1)].T
                ).astype(ml_dtypes.bfloat16),
                "cosT": cosT,
                "sinTs": sinTs,
                "tri": tri,
                "ones": ones,
                "ones_row": np.ones((1, 128), dtype=ml_dtypes.bfloat16),
            }
        )
    return in_maps


def kernel(x, W_qkv, W_proj):
    global LAST_RESULTS
    x = np.asarray(x, dtype=np.float32)
    W_qkv = np.asarray(W_qkv, dtype=np.float32)
    W_proj = np.asarray(W_proj, dtype=np.float32)
    assert x.shape == (B, T, DIM) and W_qkv.shape == (3 * H * D, DIM)

    if "nc" not in _CACHE:
        _CACHE["nc"] = _build_module()
    nc = _CACHE["nc"]

    in_maps = make_in_maps(x, W_qkv, W_proj)
    trace = os.environ.get("KERNEL_TRACE", "0") == "1"
    res = bass_utils.run_bass_kernel_spmd(
        nc, in_maps, core_ids=list(range(NCORES)), trace=trace
    )
    LAST_RESULTS = res
    y = np.zeros((B, T, DIM), dtype=np.float32)
    for c in range(NCORES):
        y[c // 4] += res.results[c]["y"].astype(np.float32)
    return y


# revision 21
# speedup vs baseline: 1.1024x; 1.1024x over previous
"""Causal self-attention (B=2, T=2048, dim=2048, H=16, D=128) on 8 trn2 NeuronCores.

Sharding: data-parallel over batch (2 groups of 4 cores), tensor-parallel over
heads within a group (4 heads/core).  Each core computes its heads' QKV
projection (x @ Wqkv_part^T), RoPE, causal attention, and a partial output
projection against its W_proj column block; the host sums the 4 partials per
batch element.

Schedule (single PE instruction stream, tile framework inserts semaphores):
  - startup: x window-0 and W_qkv stream in 128-row chunks so the first
    matmul issues ~2us in; window-0 q-tiles accumulate c-outer across 4 PSUM
    banks so the PE tracks the arriving chunks.
  - QKV windows carry RoPE (DVE) inline per window; rotate-half via
    SBUF-to-SBUF DMA on the sync queue.
  - attention per (head, query-window) block: S^T = k.T@q tiles, exp on
    Act, PV + ones-rowsum accumulate in PSUM; S matmuls run 3 tiles ahead
    of PV so the PE never waits on exp.  Softmax normalization is deferred:
    reciprocal (DVE) -> K=1 broadcast matmul into the (dead) rowsum bank ->
    one DVE multiply; the norm of each block is emitted inside the next
    block / proj so its PE bubble is covered by independent matmuls.
  - output projection for window w-1 is emitted after window w's attention
    blocks; y partials stored bf16 (host sums in f32).
"""

import os

import numpy as np
import ml_dtypes

import concourse.bass as bass
import concourse.bacc as bacc
import concourse.tile as tile
import concourse.mybir as mybir
from concourse import bass_utils

BF16 = mybir.dt.bfloat16
F32 = mybir.dt.float32

B, T, DIM = 2, 2048, 2048
H, D = 16, 128
HL = 4                   # heads per core
NCORES = 8
E = 3 * HL * D           # 1536 = per-core qkv output rows
NCHUNK = DIM // 128      # 16 contraction chunks
NW = T // 512            # 4 query windows
NTT = T // 128           # 16 token tiles
SCALE = 1.0 / float(np.sqrt(D))

_CACHE = {}
LAST_RESULTS = None


def _build_module():
    nc = bacc.Bacc("TRN2", target_bir_lowering=False, debug=False)
    xT = nc.dram_tensor("xT", (DIM, T), BF16, kind="ExternalInput")
    wqkvT = nc.dram_tensor("wqkvT", (DIM, E), BF16, kind="ExternalInput")
    wpT = nc.dram_tensor("wpT", (HL * D, DIM), BF16, kind="ExternalInput")
    cosT = nc.dram_tensor("cosT", (D, T), BF16, kind="ExternalInput")
    sinTs = nc.dram_tensor("sinTs", (D, T), BF16, kind="ExternalInput")
    tri = nc.dram_tensor("tri", (128, 128), BF16, kind="ExternalInput")
    ones = nc.dram_tensor("ones", (128, 1), BF16, kind="ExternalInput")
    ones_row = nc.dram_tensor("ones_row", (1, 128), BF16, kind="ExternalInput")
    y = nc.dram_tensor("y", (T, DIM), BF16, kind="ExternalOutput")

    Exp = mybir.ActivationFunctionType.Exp

    xT_v = xT.rearrange("(c p) t -> p c t", p=128)
    wqkvT_v = wqkvT.rearrange("(c p) e -> p c e", p=128)

    with tile.TileContext(nc) as tc:
        with (
            tc.tile_pool(name="const", bufs=1) as cpool,
            tc.tile_pool(name="xp", bufs=2) as xpool,
            tc.tile_pool(name="rotp", bufs=4) as rotpool,
            tc.tile_pool(name="ptp", bufs=4) as ptpool,
            tc.tile_pool(name="yp", bufs=4) as ypool,
            tc.tile_pool(name="rcpp", bufs=2) as rcppool,
            tc.tile_pool(name="bcp", bufs=2) as bcpool,
            tc.tile_pool(name="ps", bufs=3, space="PSUM") as pspool,
            tc.tile_pool(name="otp", bufs=3, space="PSUM") as otpool,
            tc.tile_pool(name="rsp", bufs=2, space="PSUM") as rspool,
        ):
            # persistent SBUF
            w_sb = cpool.tile([128, NCHUNK, E], BF16, tag="w")
            wp_sb = cpool.tile([128, HL, DIM], BF16, tag="wp")
            cos_sb = cpool.tile([128, T], BF16, tag="cos")
            sin_sb = cpool.tile([128, T], BF16, tag="sin")
            tri_sb = cpool.tile([128, 128], BF16, tag="tri")
            ones_sb = cpool.tile([128, 1], BF16, tag="ones")
            onesr_sb = cpool.tile([1, 128], BF16, tag="onesr")
            q_sb = cpool.tile([128, HL * T], BF16, tag="q")
            k_sb = cpool.tile([128, HL * T], BF16, tag="k")
            v_sb = cpool.tile([128, NTT * HL * D], BF16, tag="v")
            o_sb = cpool.tile([128, HL * T], BF16, tag="o")

            # ---- startup DMAs: window-0 x chunks interleaved with W chunks
            xb0 = xpool.tile([128, NCHUNK, 512], BF16, tag="x")
            for c in range(NCHUNK):
                nc.sync.dma_start(xb0[:, c, :], xT_v[:, c, 0:512])
                nc.sync.dma_start(w_sb[:, c, :], wqkvT_v[:, c, :])
            nc.sync.dma_start(cos_sb[:], cosT[:, :])
            nc.sync.dma_start(sin_sb[:], sinTs[:, :])
            nc.sync.dma_start(tri_sb[:], tri[:, :])
            nc.sync.dma_start(ones_sb[:], ones[:, :])
            nc.sync.dma_start(onesr_sb[:], ones_row[:, :])

            def rope(dst, h, w):
                """RoPE in place on dst[:, h*T + w*512 : ...+512] (d on partitions)."""
                sl = slice(h * T + w * 512, h * T + (w + 1) * 512)
                ws = slice(w * 512, (w + 1) * 512)
                rot = rotpool.tile([128, 512], BF16, tag="rot")
                nc.sync.dma_start(rot[0:64, :], dst[64:128, sl])
                nc.sync.dma_start(rot[64:128, :], dst[0:64, sl])
                nc.vector.tensor_mul(rot[:], rot[:], sin_sb[:, ws])
                nc.vector.tensor_mul(dst[:, sl], dst[:, sl], cos_sb[:, ws])
                nc.vector.tensor_add(dst[:, sl], dst[:, sl], rot[:])

            # ---- window-0 QKV.  Pass A: first 3 q tiles, c-outer over 3
            # concurrent PSUM groups so the PE consumes W/x chunks as they land.
            psA = []
            for j in range(3):
                ps = pspool.tile([128, 512], F32, tag="ps")
                psA.append(ps)
            for c in range(NCHUNK):
                for j in range(3):
                    nc.tensor.matmul(
                        psA[j][:],
                        w_sb[:, c, j * 128 : (j + 1) * 128],
                        xb0[:, c, :],
                        start=(c == 0),
                        stop=(c == NCHUNK - 1),
                    )
            for j in range(3):
                nc.scalar.copy(q_sb[:, j * T : j * T + 512], psA[j][:])
                rope(q_sb, j, 0)
            # Pass B: q3 + k tiles then v tiles (weights fully resident by now).
            for grp, j, dst in (
                [(0, 3, q_sb)] + [(1, j, k_sb) for j in range(HL)]
            ):
                ps = pspool.tile([128, 512], F32, tag="ps")
                base = grp * 512 + j * 128
                for c in range(NCHUNK):
                    nc.tensor.matmul(
                        ps[:],
                        w_sb[:, c, base : base + 128],
                        xb0[:, c, :],
                        start=(c == 0),
                        stop=(c == NCHUNK - 1),
                    )
                nc.scalar.copy(dst[:, j * T : j * T + 512], ps[:])
                rope(dst, j, 0)
            for ttl in range(4):
                ps = pspool.tile([128, 512], F32, tag="ps")
                for c in range(NCHUNK):
                    nc.tensor.matmul(
                        ps[:],
                        xb0[:, c, ttl * 128 : (ttl + 1) * 128],
                        w_sb[:, c, 1024:1536],
                        start=(c == 0),
                        stop=(c == NCHUNK - 1),
                    )
                nc.scalar.copy(v_sb[:, ttl * 512 : (ttl + 1) * 512], ps[:])

            # ---- windows 1..3 QKV + inline RoPE
            for w in range(1, NW):
                xb = xpool.tile([128, NCHUNK, 512], BF16, tag="x")
                nc.sync.dma_start(xb[:], xT_v[:, :, w * 512 : (w + 1) * 512])
                for grp, dst in ((0, q_sb), (1, k_sb)):
                    for j in range(HL):
                        ps = pspool.tile([128, 512], F32, tag="ps")
                        base = grp * 512 + j * 128
                        for c in range(NCHUNK):
                            nc.tensor.matmul(
                                ps[:],
                                w_sb[:, c, base : base + 128],
                                xb[:, c, :],
                                start=(c == 0),
                                stop=(c == NCHUNK - 1),
                            )
                        nc.scalar.copy(
                            dst[:, j * T + w * 512 : j * T + (w + 1) * 512], ps[:]
                        )
                for ttl in range(4):
                    ttg = w * 4 + ttl
                    ps = pspool.tile([128, 512], F32, tag="ps")
                    for c in range(NCHUNK):
                        nc.tensor.matmul(
                            ps[:],
                            xb[:, c, ttl * 128 : (ttl + 1) * 128],
                            w_sb[:, c, 1024:1536],
                            start=(c == 0),
                            stop=(c == NCHUNK - 1),
                        )
                    nc.scalar.copy(v_sb[:, ttg * 512 : (ttg + 1) * 512], ps[:])
                if w == 1:
                    nc.sync.dma_start(
                        wp_sb[:], wpT.rearrange("(h p) n -> p h n", p=128)
                    )

            # RoPE for windows 1..3 is staged one attention window ahead
            # (emitted inside the attention loop below): keeps the QKV
            # windows free of concurrent DVE/DMA traffic (multi-engine
            # overlap trips the PE duty throttle) without a bulk DVE
            # backlog at attention start.

            # ---- attention + proj, software pipelined -------------------
            def emit_norm(pend):
                """Deferred softmax normalization of a finished block."""
                h, w, oT, rsb = pend
                rcp = rcppool.tile([1, 512], BF16, tag="rcp")
                with nc.allow_low_precision(reason="bf16 softmax denom, ~0.4% rel"):
                    nc.vector.reciprocal(rcp[:], rsb[0:1, :])
                # broadcast rcp across partitions on gpsimd: keeps the chain
                # off PE/Act/DVE (an Act-side copy queues behind the exp
                # backlog and stalls the oT bank rotation)
                bc = bcpool.tile([128, 512], BF16, tag="bc")
                nc.gpsimd.partition_broadcast(bc[:], rcp[:], channels=128)
                nc.vector.tensor_mul(
                    o_sb[:, h * T + w * 512 : h * T + (w + 1) * 512], oT[:], bc[:]
                )

            def attn_block(h, w, pending):
                hq = h * T
                nkt = 4 * w + 4
                oT = otpool.tile([128, 512], F32, tag="ot")
                rsb = rspool.tile([1, 512], F32, tag="rs")
                pts = [None] * nkt
                geom = []
                for kt in range(nkt):
                    if kt < 4 * w:
                        geom.append((512 * w, 512, False))
                    else:
                        geom.append((128 * kt, 512 * (w + 1) - 128 * kt, True))

                def emit_S(kt):
                    q0, n, diag = geom[kt]
                    st = pspool.tile([128, 512], F32, tag="ps")
                    nc.tensor.matmul(
                        st[:, :n],
                        k_sb[:, hq + kt * 128 : hq + (kt + 1) * 128],
                        q_sb[:, hq + q0 : hq + q0 + n],
                        start=True,
                        stop=True,
                    )
                    pt = ptpool.tile([128, 512], BF16, tag="pt")
                    nc.scalar.activation(pt[:, :n], st[:, :n], Exp, bias=0.0, scale=SCALE)
                    if diag:
                        nc.vector.tensor_mul(pt[:, 0:128], pt[:, 0:128], tri_sb[:])
                    pts[kt] = pt

                for kt in range(min(3, nkt)):
                    emit_S(kt)
                while len(pending) >= 2:
                    emit_norm(pending.pop(0))
                for kt in range(nkt):
                    if kt + 3 < nkt:
                        emit_S(kt + 3)
                    q0, n, diag = geom[kt]
                    off = q0 - 512 * w
                    pt = pts[kt]
                    nc.tensor.matmul(
                        oT[:, off:512],
                        v_sb[:, kt * 512 + h * 128 : kt * 512 + (h + 1) * 128],
                        pt[:, :n],
                        start=(kt == 0),
                        stop=(kt == nkt - 1),
                    )
                    nc.tensor.matmul(
                        rsb[0:1, off:512],
                        ones_sb[:],
                        pt[:, :n],
                        start=(kt == 0),
                        stop=(kt == nkt - 1),
                    )
                pending.append((h, w, oT, rsb))

            def proj_window(pw, pending):
                for tt in range(4 * pw, 4 * pw + 4):
                    for nw2 in range(DIM // 512):
                        yps = pspool.tile([128, 512], F32, tag="ps")
                        for hh in range(HL):
                            nc.tensor.matmul(
                                yps[:],
                                o_sb[:, hh * T + tt * 128 : hh * T + (tt + 1) * 128],
                                wp_sb[:, hh, nw2 * 512 : (nw2 + 1) * 512],
                                start=(hh == 0),
                                stop=(hh == HL - 1),
                            )
                        if pending:
                            emit_norm(pending.pop(0))
                        ysb = ypool.tile([128, 512], BF16, tag="y")
                        nc.vector.tensor_copy(ysb[:], yps[:])
                        nc.sync.dma_start(
                            y[tt * 128 : (tt + 1) * 128, nw2 * 512 : (nw2 + 1) * 512],
                            ysb[:],
                        )

            pending = []
            for w in range(NW):
                for h in range(HL):
                    attn_block(h, w, pending)
                    # stage next window's RoPE for this head (DVE work rides
                    # under this window's attention matmuls)
                    if w + 1 < NW:
                        rope(q_sb, h, w + 1)
                        rope(k_sb, h, w + 1)
                if w >= 1:
                    proj_window(w - 1, pending)
            proj_window(NW - 1, pending)
            assert not pending

    nc.compile()
    return nc


def _rope_tables():
    inv_freq = (
        1.0 / (10000.0 ** (np.arange(0, D, 2, dtype=np.float32) / np.float32(D)))
    ).astype(np.float32)
    tpos = np.arange(T, dtype=np.float32)
    freqs = tpos[:, None] * inv_freq[None, :]
    emb = np.concatenate([freqs, freqs], axis=1)  # (T, D)
    cos = np.cos(emb).astype(np.float32)
    sin = np.sin(emb).astype(np.float32)
    cosT = np.ascontiguousarray(cos.T)  # (D, T)
    sinTs = np.ascontiguousarray(sin.T)
    sinTs[0:64] *= -1.0  # fold rotate_half sign
    return (
        cosT.astype(ml_dtypes.bfloat16),
        sinTs.astype(ml_dtypes.bfloat16),
    )


def make_in_maps(x, W_qkv, W_proj):
    cosT, sinTs = _rope_tables()
    tri = (np.arange(128)[None, :] >= np.arange(128)[:, None]).astype(
        ml_dtypes.bfloat16
    )
    tri = np.ascontiguousarray(tri)
    ones = np.ones((128, 1), dtype=ml_dtypes.bfloat16)
    in_maps = []
    for c in range(NCORES):
        b, g = divmod(c, 4)
        Wq = W_qkv[512 * g : 512 * (g + 1)]
        Wk = W_qkv[2048 + 512 * g : 2048 + 512 * (g + 1)]
        Wv = W_qkv[4096 + 512 * g : 4096 + 512 * (g + 1)]
        Wc = np.concatenate([Wq, Wk, Wv], axis=0)  # (1536, 2048)
        in_maps.append(
            {
                "xT": np.ascontiguousarray(x[b].T).astype(ml_dtypes.bfloat16),
                "wqkvT": np.ascontiguousarray(Wc.T).astype(ml_dtypes.bfloat16),
                "wpT": np.ascontiguousarray(
                    W_proj[:, 512 * g : 512 * (g + 1)].T
                ).astype(ml_dtypes.bfloat16),
                "cosT": cosT,
                "sinTs": sinTs,
                "tri": tri,
                "ones": ones,
                "ones_row": np.ones((1, 128), dtype=ml_dtypes.bfloat16),
            }
        )
    return in_maps


def kernel(x, W_qkv, W_proj):
    global LAST_RESULTS
    x = np.asarray(x, dtype=np.float32)
    W_qkv = np.asarray(W_qkv, dtype=np.float32)
    W_proj = np.asarray(W_proj, dtype=np.float32)
    assert x.shape == (B, T, DIM) and W_qkv.shape == (3 * H * D, DIM)

    if "nc" not in _CACHE:
        _CACHE["nc"] = _build_module()
    nc = _CACHE["nc"]

    in_maps = make_in_maps(x, W_qkv, W_proj)
    trace = os.environ.get("KERNEL_TRACE", "0") == "1"
    res = bass_utils.run_bass_kernel_spmd(
        nc, in_maps, core_ids=list(range(NCORES)), trace=trace
    )
    LAST_RESULTS = res
    y = np.zeros((B, T, DIM), dtype=np.float32)
    for c in range(NCORES):
        y[c // 4] += res.results[c]["y"].astype(np.float32)
    return y


# revision 24
# speedup vs baseline: 1.1085x; 1.0056x over previous
"""Causal self-attention (B=2, T=2048, dim=2048, H=16, D=128) on 8 trn2 NeuronCores.

Sharding: data-parallel over batch (2 groups of 4 cores), tensor-parallel over
heads within a group (4 heads/core).  Each core computes its heads' QKV
projection (x @ Wqkv_part^T), RoPE, causal attention, and a partial output
projection against its W_proj column block; the host sums the 4 partials per
batch element.

Schedule (single PE instruction stream, tile framework inserts semaphores):
  - startup: x window-0 and W_qkv stream in 128-row chunks so the first
    matmul issues ~2us in; window-0 q-tiles accumulate c-outer across 4 PSUM
    banks so the PE tracks the arriving chunks.
  - QKV windows carry RoPE (DVE) inline per window; rotate-half via
    SBUF-to-SBUF DMA on the sync queue.
  - attention per (head, query-window) block: S^T = k.T@q tiles, exp on
    Act, PV + ones-rowsum accumulate in PSUM; S matmuls run 3 tiles ahead
    of PV so the PE never waits on exp.  Softmax normalization is deferred:
    reciprocal (DVE) -> K=1 broadcast matmul into the (dead) rowsum bank ->
    one DVE multiply; the norm of each block is emitted inside the next
    block / proj so its PE bubble is covered by independent matmuls.
  - output projection for window w-1 is emitted after window w's attention
    blocks; y partials stored bf16 (host sums in f32).
"""

import os

import numpy as np
import ml_dtypes

import concourse.bass as bass
import concourse.bacc as bacc
import concourse.tile as tile
import concourse.mybir as mybir
from concourse import bass_utils

BF16 = mybir.dt.bfloat16
F32 = mybir.dt.float32

B, T, DIM = 2, 2048, 2048
H, D = 16, 128
HL = 4                   # heads per core
NCORES = 8
E = 3 * HL * D           # 1536 = per-core qkv output rows
NCHUNK = DIM // 128      # 16 contraction chunks
NW = T // 512            # 4 query windows
NTT = T // 128           # 16 token tiles
SCALE = 1.0 / float(np.sqrt(D))

_CACHE = {}
LAST_RESULTS = None


def _build_module():
    nc = bacc.Bacc("TRN2", target_bir_lowering=False, debug=False)
    xT = nc.dram_tensor("xT", (DIM, T), BF16, kind="ExternalInput")
    wqkvT = nc.dram_tensor("wqkvT", (DIM, E), BF16, kind="ExternalInput")
    wpT = nc.dram_tensor("wpT", (HL * D, DIM), BF16, kind="ExternalInput")
    cosT = nc.dram_tensor("cosT", (D, T), BF16, kind="ExternalInput")
    sinTs = nc.dram_tensor("sinTs", (D, T), BF16, kind="ExternalInput")
    tri = nc.dram_tensor("tri", (128, 128), BF16, kind="ExternalInput")
    ones = nc.dram_tensor("ones", (128, 1), BF16, kind="ExternalInput")
    ones_row = nc.dram_tensor("ones_row", (1, 128), BF16, kind="ExternalInput")
    y = nc.dram_tensor("y", (T, DIM), BF16, kind="ExternalOutput")

    Exp = mybir.ActivationFunctionType.Exp

    xT_v = xT.rearrange("(c p) t -> p c t", p=128)
    wqkvT_v = wqkvT.rearrange("(c p) e -> p c e", p=128)

    with tile.TileContext(nc) as tc:
        with (
            tc.tile_pool(name="const", bufs=1) as cpool,
            tc.tile_pool(name="xp", bufs=2) as xpool,
            tc.tile_pool(name="rotp", bufs=4) as rotpool,
            tc.tile_pool(name="ptp", bufs=4) as ptpool,
            tc.tile_pool(name="yp", bufs=4) as ypool,
            tc.tile_pool(name="rcpp", bufs=2) as rcppool,
            tc.tile_pool(name="bcp", bufs=2) as bcpool,
            tc.tile_pool(name="ps", bufs=3, space="PSUM") as pspool,
            tc.tile_pool(name="otp", bufs=3, space="PSUM") as otpool,
            tc.tile_pool(name="rsp", bufs=2, space="PSUM") as rspool,
        ):
            # persistent SBUF
            w_sb = cpool.tile([128, NCHUNK, E], BF16, tag="w")
            wp_sb = cpool.tile([128, HL, DIM], BF16, tag="wp")
            cos_sb = cpool.tile([128, T], BF16, tag="cos")
            sin_sb = cpool.tile([128, T], BF16, tag="sin")
            tri_sb = cpool.tile([128, 128], BF16, tag="tri")
            ones_sb = cpool.tile([128, 1], BF16, tag="ones")
            onesr_sb = cpool.tile([1, 128], BF16, tag="onesr")
            q_sb = cpool.tile([128, HL * T], BF16, tag="q")
            k_sb = cpool.tile([128, HL * T], BF16, tag="k")
            v_sb = cpool.tile([128, NTT * HL * D], BF16, tag="v")
            o_sb = cpool.tile([128, HL * T], BF16, tag="o")

            # ---- startup DMAs: window-0 x chunks interleaved with W chunks
            xb0 = xpool.tile([128, NCHUNK, 512], BF16, tag="x")
            for c in range(NCHUNK):
                nc.sync.dma_start(xb0[:, c, :], xT_v[:, c, 0:512])
                nc.sync.dma_start(w_sb[:, c, :], wqkvT_v[:, c, :])
            nc.sync.dma_start(cos_sb[:], cosT[:, :])
            nc.sync.dma_start(sin_sb[:], sinTs[:, :])
            nc.sync.dma_start(tri_sb[:], tri[:, :])
            nc.sync.dma_start(ones_sb[:], ones[:, :])
            nc.sync.dma_start(onesr_sb[:], ones_row[:, :])

            def rope(dst, h, w):
                """RoPE in place on dst[:, h*T + w*512 : ...+512] (d on partitions)."""
                sl = slice(h * T + w * 512, h * T + (w + 1) * 512)
                ws = slice(w * 512, (w + 1) * 512)
                rot = rotpool.tile([128, 512], BF16, tag="rot")
                nc.sync.dma_start(rot[0:64, :], dst[64:128, sl])
                nc.sync.dma_start(rot[64:128, :], dst[0:64, sl])
                nc.vector.tensor_mul(rot[:], rot[:], sin_sb[:, ws])
                nc.vector.tensor_mul(dst[:, sl], dst[:, sl], cos_sb[:, ws])
                nc.vector.tensor_add(dst[:, sl], dst[:, sl], rot[:])

            # ---- window-0 QKV.  Pass A: first 3 q tiles, c-outer over 3
            # concurrent PSUM groups so the PE consumes W/x chunks as they land.
            psA = []
            for j in range(3):
                ps = pspool.tile([128, 512], F32, tag="ps")
                psA.append(ps)
            for c in range(NCHUNK):
                for j in range(3):
                    nc.tensor.matmul(
                        psA[j][:],
                        w_sb[:, c, j * 128 : (j + 1) * 128],
                        xb0[:, c, :],
                        start=(c == 0),
                        stop=(c == NCHUNK - 1),
                    )
            for j in range(3):
                nc.scalar.copy(q_sb[:, j * T : j * T + 512], psA[j][:])
                rope(q_sb, j, 0)
            # Pass B: q3 + k tiles then v tiles (weights fully resident by now).
            for grp, j, dst in (
                [(0, 3, q_sb)] + [(1, j, k_sb) for j in range(HL)]
            ):
                ps = pspool.tile([128, 512], F32, tag="ps")
                base = grp * 512 + j * 128
                for c in range(NCHUNK):
                    nc.tensor.matmul(
                        ps[:],
                        w_sb[:, c, base : base + 128],
                        xb0[:, c, :],
                        start=(c == 0),
                        stop=(c == NCHUNK - 1),
                    )
                nc.scalar.copy(dst[:, j * T : j * T + 512], ps[:])
                rope(dst, j, 0)
            for ttl in range(4):
                ps = pspool.tile([128, 512], F32, tag="ps")
                for c in range(NCHUNK):
                    nc.tensor.matmul(
                        ps[:],
                        xb0[:, c, ttl * 128 : (ttl + 1) * 128],
                        w_sb[:, c, 1024:1536],
                        start=(c == 0),
                        stop=(c == NCHUNK - 1),
                    )
                nc.scalar.copy(v_sb[:, ttl * 512 : (ttl + 1) * 512], ps[:])

            # ---- windows 1..3 QKV + inline RoPE
            for w in range(1, NW):
                xb = xpool.tile([128, NCHUNK, 512], BF16, tag="x")
                nc.sync.dma_start(xb[:], xT_v[:, :, w * 512 : (w + 1) * 512])
                for grp, dst in ((0, q_sb), (1, k_sb)):
                    for j in range(HL):
                        ps = pspool.tile([128, 512], F32, tag="ps")
                        base = grp * 512 + j * 128
                        for c in range(NCHUNK):
                            nc.tensor.matmul(
                                ps[:],
                                w_sb[:, c, base : base + 128],
                                xb[:, c, :],
                                start=(c == 0),
                                stop=(c == NCHUNK - 1),
                            )
                        nc.scalar.copy(
                            dst[:, j * T + w * 512 : j * T + (w + 1) * 512], ps[:]
                        )
                for ttl in range(4):
                    ttg = w * 4 + ttl
                    ps = pspool.tile([128, 512], F32, tag="ps")
                    for c in range(NCHUNK):
                        nc.tensor.matmul(
                            ps[:],
                            xb[:, c, ttl * 128 : (ttl + 1) * 128],
                            w_sb[:, c, 1024:1536],
                            start=(c == 0),
                            stop=(c == NCHUNK - 1),
                        )
                    nc.scalar.copy(v_sb[:, ttg * 512 : (ttg + 1) * 512], ps[:])
                if w == 1:
                    nc.sync.dma_start(
                        wp_sb[:], wpT.rearrange("(h p) n -> p h n", p=128)
                    )

            # RoPE for windows 1..3 is staged one attention window ahead
            # (emitted inside the attention loop below): keeps the QKV
            # windows free of concurrent DVE/DMA traffic (multi-engine
            # overlap trips the PE duty throttle) without a bulk DVE
            # backlog at attention start.

            # ---- attention + proj, software pipelined -------------------
            def emit_norm(pend):
                """Deferred softmax normalization of a finished block."""
                h, w, oT, rsb = pend
                rcp = rcppool.tile([1, 512], BF16, tag="rcp")
                with nc.allow_low_precision(reason="bf16 softmax denom, ~0.4% rel"):
                    nc.vector.reciprocal(rcp[:], rsb[0:1, :])
                # broadcast rcp across partitions on gpsimd: keeps the chain
                # off PE/Act/DVE (an Act-side copy queues behind the exp
                # backlog and stalls the oT bank rotation)
                bc = bcpool.tile([128, 512], BF16, tag="bc")
                nc.gpsimd.partition_broadcast(bc[:], rcp[:], channels=128)
                nc.vector.tensor_mul(
                    o_sb[:, h * T + w * 512 : h * T + (w + 1) * 512], oT[:], bc[:]
                )

            def attn_block(h, w, pending):
                hq = h * T
                nkt = 4 * w + 4
                oT = otpool.tile([128, 512], F32, tag="ot")
                rsb = rspool.tile([1, 512], F32, tag="rs")
                pts = [None] * nkt
                geom = []
                for kt in range(nkt):
                    if kt < 4 * w:
                        geom.append((512 * w, 512, False))
                    else:
                        geom.append((128 * kt, 512 * (w + 1) - 128 * kt, True))

                def emit_S(kt):
                    q0, n, diag = geom[kt]
                    st = pspool.tile([128, 512], F32, tag="ps")
                    nc.tensor.matmul(
                        st[:, :n],
                        k_sb[:, hq + kt * 128 : hq + (kt + 1) * 128],
                        q_sb[:, hq + q0 : hq + q0 + n],
                        start=True,
                        stop=True,
                    )
                    pt = ptpool.tile([128, 512], BF16, tag="pt")
                    nc.scalar.activation(pt[:, :n], st[:, :n], Exp, bias=0.0, scale=SCALE)
                    if diag:
                        nc.vector.tensor_mul(pt[:, 0:128], pt[:, 0:128], tri_sb[:])
                    pts[kt] = pt

                for kt in range(min(3, nkt)):
                    emit_S(kt)
                while len(pending) >= 2:
                    emit_norm(pending.pop(0))
                for kt in range(nkt):
                    if kt + 3 < nkt:
                        emit_S(kt + 3)
                    q0, n, diag = geom[kt]
                    off = q0 - 512 * w
                    pt = pts[kt]
                    nc.tensor.matmul(
                        oT[:, off:512],
                        v_sb[:, kt * 512 + h * 128 : kt * 512 + (h + 1) * 128],
                        pt[:, :n],
                        start=(kt == 0),
                        stop=(kt == nkt - 1),
                    )
                    nc.tensor.matmul(
                        rsb[0:1, off:512],
                        ones_sb[:],
                        pt[:, :n],
                        start=(kt == 0),
                        stop=(kt == nkt - 1),
                    )
                pending.append((h, w, oT, rsb))

            def proj_window(pw, pending):
                # all outstanding norms must be emitted before any proj
                # matmul that might read their o_sb slices
                while pending:
                    emit_norm(pending.pop(0))
                for tt in range(4 * pw, 4 * pw + 4):
                    for nw2 in range(DIM // 512):
                        yps = pspool.tile([128, 512], F32, tag="ps")
                        for hh in range(HL):
                            nc.tensor.matmul(
                                yps[:],
                                o_sb[:, hh * T + tt * 128 : hh * T + (tt + 1) * 128],
                                wp_sb[:, hh, nw2 * 512 : (nw2 + 1) * 512],
                                start=(hh == 0),
                                stop=(hh == HL - 1),
                            )
                        ysb = ypool.tile([128, 512], BF16, tag="y")
                        nc.scalar.copy(ysb[:], yps[:])
                        nc.sync.dma_start(
                            y[tt * 128 : (tt + 1) * 128, nw2 * 512 : (nw2 + 1) * 512],
                            ysb[:],
                        )

            pending = []
            for w in range(NW):
                for h in range(HL):
                    attn_block(h, w, pending)
                    # stage next window's RoPE for this head (DVE work rides
                    # under this window's attention matmuls)
                    if w + 1 < NW:
                        rope(q_sb, h, w + 1)
                        rope(k_sb, h, w + 1)
                    # previous window's projection lands mid-window: its 16
                    # dense independent matmul groups cover the norm chains
                    # and rope DVE traffic of the current window's blocks
                    if w >= 1 and h == 1:
                        proj_window(w - 1, pending)
            proj_window(NW - 1, pending)
            assert not pending

    nc.compile()
    return nc


def _rope_tables():
    inv_freq = (
        1.0 / (10000.0 ** (np.arange(0, D, 2, dtype=np.float32) / np.float32(D)))
    ).astype(np.float32)
    tpos = np.arange(T, dtype=np.float32)
    freqs = tpos[:, None] * inv_freq[None, :]
    emb = np.concatenate([freqs, freqs], axis=1)  # (T, D)
    cos = np.cos(emb).astype(np.float32)
    sin = np.sin(emb).astype(np.float32)
    cosT = np.ascontiguousarray(cos.T)  # (D, T)
    sinTs = np.ascontiguousarray(sin.T)
    sinTs[0:64] *= -1.0  # fold rotate_half sign
    return (
        cosT.astype(ml_dtypes.bfloat16),
        sinTs.astype(ml_dtypes.bfloat16),
    )


def make_in_maps(x, W_qkv, W_proj):
    cosT, sinTs = _rope_tables()
    tri = (np.arange(128)[None, :] >= np.arange(128)[:, None]).astype(
        ml_dtypes.bfloat16
    )
    tri = np.ascontiguousarray(tri)
    ones = np.ones((128, 1), dtype=ml_dtypes.bfloat16)
    in_maps = []
    for c in range(NCORES):
        b, g = divmod(c, 4)
        Wq = W_qkv[512 * g : 512 * (g + 1)]
        Wk = W_qkv[2048 + 512 * g : 2048 + 512 * (g + 1)]
        Wv = W_qkv[4096 + 512 * g : 4096 + 512 * (g + 1)]
        Wc = np.concatenate([Wq, Wk, Wv], axis=0)  # (1536, 2048)
        in_maps.append(
            {
                "xT": np.ascontiguousarray(x[b].T).astype(ml_dtypes.bfloat16),
                "wqkvT": np.ascontiguousarray(Wc.T).astype(ml_dtypes.bfloat16),
                "wpT": np.ascontiguousarray(
                    W_proj[:, 512 * g : 512 * (g + 1)].T
                ).astype(ml_dtypes.bfloat16),
                "cosT": cosT,
                "sinTs": sinTs,
                "tri": tri,
                "ones": ones,
                "ones_row": np.ones((1, 128), dtype=ml_dtypes.bfloat16),
            }
        )
    return in_maps


def kernel(x, W_qkv, W_proj):
    global LAST_RESULTS
    x = np.asarray(x, dtype=np.float32)
    W_qkv = np.asarray(W_qkv, dtype=np.float32)
    W_proj = np.asarray(W_proj, dtype=np.float32)
    assert x.shape == (B, T, DIM) and W_qkv.shape == (3 * H * D, DIM)

    if "nc" not in _CACHE:
        _CACHE["nc"] = _build_module()
    nc = _CACHE["nc"]

    in_maps = make_in_maps(x, W_qkv, W_proj)
    trace = os.environ.get("KERNEL_TRACE", "0") == "1"
    res = bass_utils.run_bass_kernel_spmd(
        nc, in_maps, core_ids=list(range(NCORES)), trace=trace
    )
    LAST_RESULTS = res
    y = np.zeros((B, T, DIM), dtype=np.float32)
    for c in range(NCORES):
        y[c // 4] += res.results[c]["y"].astype(np.float32)
    return y


# revision 25
# speedup vs baseline: 1.1196x; 1.0100x over previous
"""Causal self-attention (B=2, T=2048, dim=2048, H=16, D=128) on 8 trn2 NeuronCores.

Sharding: data-parallel over batch (2 groups of 4 cores), tensor-parallel over
heads within a group (4 heads/core).  Each core computes its heads' QKV
projection (x @ Wqkv_part^T), RoPE, causal attention, and a partial output
projection against its W_proj column block; the host sums the 4 partials per
batch element.

Schedule (single PE instruction stream, tile framework inserts semaphores):
  - startup: x window-0 and W_qkv stream in 128-row chunks so the first
    matmul issues ~2us in; window-0 q-tiles accumulate c-outer across 4 PSUM
    banks so the PE tracks the arriving chunks.
  - QKV windows carry RoPE (DVE) inline per window; rotate-half via
    SBUF-to-SBUF DMA on the sync queue.
  - attention per (head, query-window) block: S^T = k.T@q tiles, exp on
    Act, PV + ones-rowsum accumulate in PSUM; S matmuls run 3 tiles ahead
    of PV so the PE never waits on exp.  Softmax normalization is deferred:
    reciprocal (DVE) -> K=1 broadcast matmul into the (dead) rowsum bank ->
    one DVE multiply; the norm of each block is emitted inside the next
    block / proj so its PE bubble is covered by independent matmuls.
  - output projection for window w-1 is emitted after window w's attention
    blocks; y partials stored bf16 (host sums in f32).
"""

import os

import numpy as np
import ml_dtypes

import concourse.bass as bass
import concourse.bacc as bacc
import concourse.tile as tile
import concourse.mybir as mybir
from concourse import bass_utils

BF16 = mybir.dt.bfloat16
F32 = mybir.dt.float32

B, T, DIM = 2, 2048, 2048
H, D = 16, 128
HL = 4                   # heads per core
NCORES = 8
E = 3 * HL * D           # 1536 = per-core qkv output rows
NCHUNK = DIM // 128      # 16 contraction chunks
NW = T // 512            # 4 query windows
NTT = T // 128           # 16 token tiles
SCALE = 1.0 / float(np.sqrt(D))

_CACHE = {}
LAST_RESULTS = None


def _build_module():
    nc = bacc.Bacc("TRN2", target_bir_lowering=False, debug=False)
    xT = nc.dram_tensor("xT", (DIM, T), BF16, kind="ExternalInput")
    wqkvT = nc.dram_tensor("wqkvT", (DIM, E), BF16, kind="ExternalInput")
    wpT = nc.dram_tensor("wpT", (HL * D, DIM), BF16, kind="ExternalInput")
    cosT = nc.dram_tensor("cosT", (D, T), BF16, kind="ExternalInput")
    sinTs = nc.dram_tensor("sinTs", (D, T), BF16, kind="ExternalInput")
    tri = nc.dram_tensor("tri", (128, 128), BF16, kind="ExternalInput")
    ones = nc.dram_tensor("ones", (128, 1), BF16, kind="ExternalInput")
    ones_row = nc.dram_tensor("ones_row", (1, 128), BF16, kind="ExternalInput")
    y = nc.dram_tensor("y", (T, DIM), BF16, kind="ExternalOutput")

    Exp = mybir.ActivationFunctionType.Exp

    xT_v = xT.rearrange("(c p) t -> p c t", p=128)
    wqkvT_v = wqkvT.rearrange("(c p) e -> p c e", p=128)

    with tile.TileContext(nc) as tc:
        with (
            tc.tile_pool(name="const", bufs=1) as cpool,
            tc.tile_pool(name="xp", bufs=2) as xpool,
            tc.tile_pool(name="rotp", bufs=4) as rotpool,
            tc.tile_pool(name="ptp", bufs=4) as ptpool,
            tc.tile_pool(name="yp", bufs=4) as ypool,
            tc.tile_pool(name="rcpp", bufs=2) as rcppool,
            tc.tile_pool(name="bcp", bufs=2) as bcpool,
            tc.tile_pool(name="ps", bufs=3, space="PSUM") as pspool,
            tc.tile_pool(name="otp", bufs=3, space="PSUM") as otpool,
            tc.tile_pool(name="rsp", bufs=2, space="PSUM") as rspool,
        ):
            # persistent SBUF
            w_sb = cpool.tile([128, NCHUNK, E], BF16, tag="w")
            wp_sb = cpool.tile([128, HL, DIM], BF16, tag="wp")
            cos_sb = cpool.tile([128, T], BF16, tag="cos")
            sin_sb = cpool.tile([128, T], BF16, tag="sin")
            tri_sb = cpool.tile([128, 128], BF16, tag="tri")
            ones_sb = cpool.tile([128, 1], BF16, tag="ones")
            onesr_sb = cpool.tile([1, 128], BF16, tag="onesr")
            q_sb = cpool.tile([128, HL * T], BF16, tag="q")
            k_sb = cpool.tile([128, HL * T], BF16, tag="k")
            v_sb = cpool.tile([128, NTT * HL * D], BF16, tag="v")
            o_sb = cpool.tile([128, HL * T], BF16, tag="o")

            # ---- startup DMAs: window-0 x chunks interleaved with W chunks
            # q-group weight chunks stream first so window-0 pass A paces on
            # x+Wq (4.2MB) instead of the full W (8.4MB); k and v groups
            # follow in coarse blocks (few dispatches — a long dispatch queue
            # delays the window-1 x load behind it)
            xb0 = xpool.tile([128, NCHUNK, 512], BF16, tag="x")
            for c in range(NCHUNK):
                nc.sync.dma_start(xb0[:, c, :], xT_v[:, c, 0:512])
                if c % 4 == 3:
                    nc.sync.dma_start(
                        w_sb[:, c - 3 : c + 1, 0:512],
                        wqkvT_v[:, c - 3 : c + 1, 0:512],
                    )
            for grp in (1, 2):
                for c0 in range(0, NCHUNK, 4):
                    nc.sync.dma_start(
                        w_sb[:, c0 : c0 + 4, grp * 512 : (grp + 1) * 512],
                        wqkvT_v[:, c0 : c0 + 4, grp * 512 : (grp + 1) * 512],
                    )
            nc.sync.dma_start(cos_sb[:], cosT[:, :])
            nc.sync.dma_start(sin_sb[:], sinTs[:, :])
            nc.sync.dma_start(tri_sb[:], tri[:, :])
            nc.sync.dma_start(ones_sb[:], ones[:, :])
            nc.sync.dma_start(onesr_sb[:], ones_row[:, :])

            def rope(dst, h, w):
                """RoPE in place on dst[:, h*T + w*512 : ...+512] (d on partitions)."""
                sl = slice(h * T + w * 512, h * T + (w + 1) * 512)
                ws = slice(w * 512, (w + 1) * 512)
                rot = rotpool.tile([128, 512], BF16, tag="rot")
                nc.sync.dma_start(rot[0:64, :], dst[64:128, sl])
                nc.sync.dma_start(rot[64:128, :], dst[0:64, sl])
                nc.vector.tensor_mul(rot[:], rot[:], sin_sb[:, ws])
                nc.vector.tensor_mul(dst[:, sl], dst[:, sl], cos_sb[:, ws])
                nc.vector.tensor_add(dst[:, sl], dst[:, sl], rot[:])

            # ---- window-0 QKV.  Pass A: first 3 q tiles, c-outer over 3
            # concurrent PSUM groups so the PE consumes W/x chunks as they land.
            psA = []
            for j in range(3):
                ps = pspool.tile([128, 512], F32, tag="ps")
                psA.append(ps)
            for c in range(NCHUNK):
                for j in range(3):
                    nc.tensor.matmul(
                        psA[j][:],
                        w_sb[:, c, j * 128 : (j + 1) * 128],
                        xb0[:, c, :],
                        start=(c == 0),
                        stop=(c == NCHUNK - 1),
                    )
            for j in range(3):
                nc.scalar.copy(q_sb[:, j * T : j * T + 512], psA[j][:])
                rope(q_sb, j, 0)
            # Pass B: q3 + k tiles then v tiles (weights fully resident by now).
            for grp, j, dst in (
                [(0, 3, q_sb)] + [(1, j, k_sb) for j in range(HL)]
            ):
                ps = pspool.tile([128, 512], F32, tag="ps")
                base = grp * 512 + j * 128
                for c in range(NCHUNK):
                    nc.tensor.matmul(
                        ps[:],
                        w_sb[:, c, base : base + 128],
                        xb0[:, c, :],
                        start=(c == 0),
                        stop=(c == NCHUNK - 1),
                    )
                nc.scalar.copy(dst[:, j * T : j * T + 512], ps[:])
                rope(dst, j, 0)
            for ttl in range(4):
                ps = pspool.tile([128, 512], F32, tag="ps")
                for c in range(NCHUNK):
                    nc.tensor.matmul(
                        ps[:],
                        xb0[:, c, ttl * 128 : (ttl + 1) * 128],
                        w_sb[:, c, 1024:1536],
                        start=(c == 0),
                        stop=(c == NCHUNK - 1),
                    )
                nc.scalar.copy(v_sb[:, ttl * 512 : (ttl + 1) * 512], ps[:])

            # ---- windows 1..3 QKV + inline RoPE
            for w in range(1, NW):
                xb = xpool.tile([128, NCHUNK, 512], BF16, tag="x")
                nc.sync.dma_start(xb[:], xT_v[:, :, w * 512 : (w + 1) * 512])
                for grp, dst in ((0, q_sb), (1, k_sb)):
                    for j in range(HL):
                        ps = pspool.tile([128, 512], F32, tag="ps")
                        base = grp * 512 + j * 128
                        for c in range(NCHUNK):
                            nc.tensor.matmul(
                                ps[:],
                                w_sb[:, c, base : base + 128],
                                xb[:, c, :],
                                start=(c == 0),
                                stop=(c == NCHUNK - 1),
                            )
                        nc.scalar.copy(
                            dst[:, j * T + w * 512 : j * T + (w + 1) * 512], ps[:]
                        )
                for ttl in range(4):
                    ttg = w * 4 + ttl
                    ps = pspool.tile([128, 512], F32, tag="ps")
                    for c in range(NCHUNK):
                        nc.tensor.matmul(
                            ps[:],
                            xb[:, c, ttl * 128 : (ttl + 1) * 128],
                            w_sb[:, c, 1024:1536],
                            start=(c == 0),
                            stop=(c == NCHUNK - 1),
                        )
                    nc.scalar.copy(v_sb[:, ttg * 512 : (ttg + 1) * 512], ps[:])
                if w == 1:
                    nc.sync.dma_start(
                        wp_sb[:], wpT.rearrange("(h p) n -> p h n", p=128)
                    )

            # RoPE for windows 1..3 is staged one attention window ahead
            # (emitted inside the attention loop below): keeps the QKV
            # windows free of concurrent DVE/DMA traffic (multi-engine
            # overlap trips the PE duty throttle) without a bulk DVE
            # backlog at attention start.

            # ---- attention + proj, software pipelined -------------------
            def emit_norm(pend):
                """Deferred softmax normalization of a finished block."""
                h, w, oT, rsb = pend
                rcp = rcppool.tile([1, 512], BF16, tag="rcp")
                with nc.allow_low_precision(reason="bf16 softmax denom, ~0.4% rel"):
                    nc.vector.reciprocal(rcp[:], rsb[0:1, :])
                # broadcast rcp across partitions on gpsimd: keeps the chain
                # off PE/Act/DVE (an Act-side copy queues behind the exp
                # backlog and stalls the oT bank rotation)
                bc = bcpool.tile([128, 512], BF16, tag="bc")
                nc.gpsimd.partition_broadcast(bc[:], rcp[:], channels=128)
                nc.vector.tensor_mul(
                    o_sb[:, h * T + w * 512 : h * T + (w + 1) * 512], oT[:], bc[:]
                )

            def attn_block(h, w, pending):
                hq = h * T
                nkt = 4 * w + 4
                oT = otpool.tile([128, 512], F32, tag="ot")
                rsb = rspool.tile([1, 512], F32, tag="rs")
                pts = [None] * nkt
                geom = []
                for kt in range(nkt):
                    if kt < 4 * w:
                        geom.append((512 * w, 512, False))
                    else:
                        geom.append((128 * kt, 512 * (w + 1) - 128 * kt, True))

                def emit_S(kt):
                    q0, n, diag = geom[kt]
                    st = pspool.tile([128, 512], F32, tag="ps")
                    nc.tensor.matmul(
                        st[:, :n],
                        k_sb[:, hq + kt * 128 : hq + (kt + 1) * 128],
                        q_sb[:, hq + q0 : hq + q0 + n],
                        start=True,
                        stop=True,
                    )
                    pt = ptpool.tile([128, 512], BF16, tag="pt")
                    nc.scalar.activation(pt[:, :n], st[:, :n], Exp, bias=0.0, scale=SCALE)
                    if diag:
                        nc.vector.tensor_mul(pt[:, 0:128], pt[:, 0:128], tri_sb[:])
                    pts[kt] = pt

                for kt in range(min(3, nkt)):
                    emit_S(kt)
                while len(pending) >= 2:
                    emit_norm(pending.pop(0))
                for kt in range(nkt):
                    if kt + 3 < nkt:
                        emit_S(kt + 3)
                    q0, n, diag = geom[kt]
                    off = q0 - 512 * w
                    pt = pts[kt]
                    nc.tensor.matmul(
                        oT[:, off:512],
                        v_sb[:, kt * 512 + h * 128 : kt * 512 + (h + 1) * 128],
                        pt[:, :n],
                        start=(kt == 0),
                        stop=(kt == nkt - 1),
                    )
                    nc.tensor.matmul(
                        rsb[0:1, off:512],
                        ones_sb[:],
                        pt[:, :n],
                        start=(kt == 0),
                        stop=(kt == nkt - 1),
                    )
                pending.append((h, w, oT, rsb))

            def proj_window(pw, pending):
                # all outstanding norms must be emitted before any proj
                # matmul that might read their o_sb slices
                while pending:
                    emit_norm(pending.pop(0))
                for tt in range(4 * pw, 4 * pw + 4):
                    for nw2 in range(DIM // 512):
                        yps = pspool.tile([128, 512], F32, tag="ps")
                        for hh in range(HL):
                            nc.tensor.matmul(
                                yps[:],
                                o_sb[:, hh * T + tt * 128 : hh * T + (tt + 1) * 128],
                                wp_sb[:, hh, nw2 * 512 : (nw2 + 1) * 512],
                                start=(hh == 0),
                                stop=(hh == HL - 1),
                            )
                        ysb = ypool.tile([128, 512], BF16, tag="y")
                        nc.scalar.copy(ysb[:], yps[:])
                        nc.sync.dma_start(
                            y[tt * 128 : (tt + 1) * 128, nw2 * 512 : (nw2 + 1) * 512],
                            ysb[:],
                        )

            pending = []
            for w in range(NW):
                for h in range(HL):
                    attn_block(h, w, pending)
                    # stage next window's RoPE for this head (DVE work rides
                    # under this window's attention matmuls)
                    if w + 1 < NW:
                        rope(q_sb, h, w + 1)
                        rope(k_sb, h, w + 1)
                    # previous window's projection lands mid-window: its 16
                    # dense independent matmul groups cover the norm chains
                    # and rope DVE traffic of the current window's blocks
                    if w >= 1 and h == 1:
                        proj_window(w - 1, pending)
            proj_window(NW - 1, pending)
            assert not pending

    nc.compile()
    return nc


def _rope_tables():
    inv_freq = (
        1.0 / (10000.0 ** (np.arange(0, D, 2, dtype=np.float32) / np.float32(D)))
    ).astype(np.float32)
    tpos = np.arange(T, dtype=np.float32)
    freqs = tpos[:, None] * inv_freq[None, :]
    emb = np.concatenate([freqs, freqs], axis=1)  # (T, D)
    cos = np.cos(emb).astype(np.float32)
    sin = np.sin(emb).astype(np.float32)
    cosT = np.ascontiguousarray(cos.T)  # (D, T)
    sinTs = np.ascontiguousarray(sin.T)
    sinTs[0:64] *= -1.0  # fold rotate_half sign
    return (
        cosT.astype(ml_dtypes.bfloat16),
        sinTs.astype(ml_dtypes.bfloat16),
    )


def make_in_maps(x, W_qkv, W_proj):
    cosT, sinTs = _rope_tables()
    tri = (np.arange(128)[None, :] >= np.arange(128)[:, None]).astype(
        ml_dtypes.bfloat16
    )
    tri = np.ascontiguousarray(tri)
    ones = np.ones((128, 1), dtype=ml_dtypes.bfloat16)
    in_maps = []
    for c in range(NCORES):
        b, g = divmod(c, 4)
        Wq = W_qkv[512 * g : 512 * (g + 1)]
        Wk = W_qkv[2048 + 512 * g : 2048 + 512 * (g + 1)]
        Wv = W_qkv[4096 + 512 * g : 4096 + 512 * (g + 1)]
        Wc = np.concatenate([Wq, Wk, Wv], axis=0)  # (1536, 2048)
        in_maps.append(
            {
                "xT": np.ascontiguousarray(x[b].T).astype(ml_dtypes.bfloat16),
                "wqkvT": np.ascontiguousarray(Wc.T).astype(ml_dtypes.bfloat16),
                "wpT": np.ascontiguousarray(
                    W_proj[:, 512 * g : 512 * (g + 1)].T
                ).astype(ml_dtypes.bfloat16),
                "cosT": cosT,
                "sinTs": sinTs,
                "tri": tri,
                "ones": ones,
                "ones_row": np.ones((1, 128), dtype=ml_dtypes.bfloat16),
            }
        )
    return in_maps


def kernel(x, W_qkv, W_proj):
    global LAST_RESULTS
    x = np.asarray(x, dtype=np.float32)
    W_qkv = np.asarray(W_qkv, dtype=np.float32)
    W_proj = np.asarray(W_proj, dtype=np.float32)
    assert x.shape == (B, T, DIM) and W_qkv.shape == (3 * H * D, DIM)

    if "nc" not in _CACHE:
        _CACHE["nc"] = _build_module()
    nc = _CACHE["nc"]

    in_maps = make_in_maps(x, W_qkv, W_proj)
    trace = os.environ.get("KERNEL_TRACE", "0") == "1"
    res = bass_utils.run_bass_kernel_spmd(
        nc, in_maps, core_ids=list(range(NCORES)), trace=trace
    )
    LAST_RESULTS = res
    y = np.zeros((B, T, DIM), dtype=np.float32)
    for c in range(NCORES):
        y[c // 4] += res.results[c]["y"].astype(np.float32)
    return y
